# revision 1
# baseline (speedup 1.0000x reference)
"""GAT layer (gnn_message_passing) on 8 trn2 NeuronCores.

Strategy (dst-sharded, no collectives):
- Each core owns a contiguous 1/8 slice of target nodes; host buckets edges by
  dst core. Within a core, owned nodes are sorted by in-degree (descending) and
  grouped into 128-node windows; node -> SBUF partition, its in-edges occupy
  "slot columns" t=0..deg-1 of that partition (degree sorting makes the
  per-window column count ~= mean degree, tiny padding).
- Per edge slot, xp[src] (128 f32) is fetched with SWDGE dma_gather from an
  HBM pair-table ([25000, 256] f32, idx = perm_pos(src)>>1, int16) built on
  device in pass-0 via xp = x @ W_lin.T. The pair trick keeps indices within
  int16 while covering all 50000 rows; the unneeded half is masked on-chip
  (sel_lo / sel_hi host masks fold parity AND slot-validity).
- Attention logits: a_s from gathered rows (DVE mult + grouped reduce with a
  replicated w_s tile), a_e from slotted edge_attr (DVE with replicated folded
  C), a_t + all scalar biases folded into a per-node column computed in pass-0
  (x @ D_ext). leaky-relu on DVE (scalar_tensor_tensor), exp on ACT. Softmax
  max-subtraction is dropped: logits are O(1) so exp is safe and softmax is
  shift-invariant.
- Messages msg = expv * xs accumulate into a PSUM window via identity-lhsT
  matmuls (PE), with expv appended as 4 extra columns -> denominator comes out
  of the same matmuls. Residual x @ W_res.T + bias via a ones-row-extended
  matmul into a second PSUM tile. out = num/denom + res.
"""
import os
import sys
from contextlib import ExitStack

sys.path.insert(0, "/opt/trn_rl_repo")

import numpy as np

N, E = 50000, 1600000
IN_F, EDGE_F, HEADS, OUT_F = 64, 16, 4, 32
NEG_SLOPE = 0.2
NCORES = 8
NODES_PC = N // NCORES            # 6250
NW = (NODES_PC + 127) // 128      # 49 windows/core
WNODES = NW * 128                 # 6272 (last window partially real)
TC_TILES = 12                     # gather-chunk size in 128-slot tiles

f32 = None  # set after bass import (lazy so numpy-only import of this file works)


def _host_preprocess(x, edge_index, edge_attr, W_lin, w_s, b_s, w_t, b_t,
                     W_edge, w_e, b_e, W_res, bias):
    """Pure index/layout work + weight folding. Returns (common, per_core)."""
    src = edge_index[0].astype(np.int64)
    dst = edge_index[1].astype(np.int64)
    deg = np.bincount(dst, minlength=N)

    # ---- weight folding (weights only; standard operator fusion) ----
    wlinT = np.ascontiguousarray(W_lin.T)                      # [64, 128]
    ws_concat = np.tile(w_s, HEADS)                            # [128]
    wsrep = np.tile(ws_concat[None, :], (128, 1)).astype(np.float32)
    C = (W_edge.reshape(HEADS, OUT_F, EDGE_F) * w_e[None, :, None]).sum(1)  # [4,16]
    crep = np.tile(C.reshape(-1)[None, :], (128, 1)).astype(np.float32)    # [128,64]
    D = (W_lin.reshape(HEADS, OUT_F, IN_F) * w_t[None, :, None]).sum(1).T  # [64,4]
    b_total = float(b_s) + float(b_t) + float(b_e)
    dext = np.vstack([D, np.full((1, HEADS), b_total, np.float32)]).astype(np.float32)
    wrese = np.vstack([W_res.T, bias[None, :]]).astype(np.float32)         # [65,128]
    ident = np.eye(128, dtype=np.float32)

    # ---- per-core schedules (common T_w across cores) ----
    cores = []
    for c in range(NCORES):
        lo = c * NODES_PC
        owned = np.arange(lo, lo + NODES_PC)
        dc = deg[owned]
        order = np.argsort(-dc, kind="stable")
        perm_owned = owned[order]
        degs_sorted = dc[order]
        # per-window max degree = first element (descending sort)
        tw = np.maximum(degs_sorted[::128][:NW], 1).astype(np.int64)
        cores.append(dict(perm_owned=perm_owned, degs_sorted=degs_sorted, tw=tw))

    T_w = np.max(np.stack([cc["tw"] for cc in cores]), axis=0)  # [NW]
    TOFF = np.concatenate([[0], np.cumsum(T_w)])                # slot col offsets
    SUMT = int(TOFF[-1])

    # chunk layout (common): per window, tiles split into chunks of <= TC_TILES
    chunks = []           # list of (w, t0, t1, icol0)
    icol = 0
    for w in range(NW):
        t = 0
        while t < T_w[w]:
            t1 = min(t + TC_TILES, int(T_w[w]))
            chunks.append((w, t, t1, icol))
            icol += (t1 - t) * 8
            t += t1 - t
    IDXCOLS = icol

    per_core = []
    for c in range(NCORES):
        cc = cores[c]
        perm_owned = cc["perm_owned"]
        rest = np.setdiff1d(np.arange(N), perm_owned, assume_unique=True)
        perm = np.concatenate([perm_owned, rest])
        perm_pos = np.empty(N, np.int64)
        perm_pos[perm] = np.arange(N)

        emask = (dst >= c * NODES_PC) & (dst < (c + 1) * NODES_PC)
        e_ids = np.nonzero(emask)[0]
        d_loc = perm_pos[dst[e_ids]]                 # 0..6249
        eorder = np.argsort(d_loc, kind="stable")
        e_s = e_ids[eorder]
        ds = d_loc[eorder]
        starts = np.searchsorted(ds, np.arange(NODES_PC))
        t_of = np.arange(len(ds)) - starts[ds]
        w_of = ds // 128
        p_of = ds % 128
        col = TOFF[w_of] + t_of

        src_rel = perm_pos[src[e_s]]
        par = (src_rel & 1).astype(np.float32)

        idx_slot = np.zeros((128, SUMT), np.int16)
        sel = np.zeros((2, 128, SUMT), np.float32)
        ea_slot = np.zeros((128, SUMT, EDGE_F), np.float32)
        idx_slot[p_of, col] = (src_rel >> 1).astype(np.int16)
        sel[0, p_of, col] = 1.0 - par
        sel[1, p_of, col] = par
        ea_slot[p_of, col] = edge_attr[e_s]

        # wrapped idx arrays per chunk: positions i=(t-t0)*128+p -> [i%16, i//16]
        idx16 = np.zeros((128, IDXCOLS), np.int16)
        for (w, t0, t1, ic0) in chunks:
            ncol = (t1 - t0) * 8
            flat = idx_slot[:, TOFF[w] + t0: TOFF[w] + t1].T.reshape(-1)  # [(t1-t0)*128]
            wrapped = flat.reshape(-1, 16).T                              # [16, ncol]
            idx16[:, ic0: ic0 + ncol] = np.tile(wrapped, (8, 1))

        xT_ext = np.empty((IN_F + 1, N), np.float32)
        xT_ext[:IN_F] = x[perm].T
        xT_ext[IN_F] = 1.0

        per_core.append(dict(
            xT=xT_ext,
            idx16=idx16,
            sel=sel,
            ea=ea_slot.reshape(128, SUMT * EDGE_F),
            perm_owned=perm_owned,
        ))

    common = dict(T_w=T_w, TOFF=TOFF, SUMT=SUMT, chunks=chunks, IDXCOLS=IDXCOLS,
                  wlinT=wlinT.astype(np.float32), dext=dext, wsrep=wsrep,
                  crep=crep, wrese=wrese, ident=ident)
    return common, per_core


def _build_program(common):
    import concourse.bass as bass
    import concourse.tile as tile
    from concourse import bacc, mybir

    f32 = mybir.dt.float32
    i16 = mybir.dt.int16
    AL = mybir.AluOpType
    SUMT, IDXCOLS = common["SUMT"], common["IDXCOLS"]
    T_w, TOFF, chunks = common["T_w"], common["TOFF"], common["chunks"]

    nc = bacc.Bacc("TRN2", target_bir_lowering=False, debug=False,
                   num_devices=NCORES, num_swdge_queues=4)

    xT_d = nc.dram_tensor("xT", [IN_F + 1, N], f32, kind="ExternalInput")
    idx_d = nc.dram_tensor("idx16", [128, IDXCOLS], i16, kind="ExternalInput")
    sel_d = nc.dram_tensor("sel", [2, 128, SUMT], f32, kind="ExternalInput")
    ea_d = nc.dram_tensor("ea", [128, SUMT * EDGE_F], f32, kind="ExternalInput")
    wlin_d = nc.dram_tensor("wlinT", [IN_F, 128], f32, kind="ExternalInput")
    dext_d = nc.dram_tensor("dext", [IN_F + 1, HEADS], f32, kind="ExternalInput")
    wsrep_d = nc.dram_tensor("wsrep", [128, 128], f32, kind="ExternalInput")
    crep_d = nc.dram_tensor("crep", [128, HEADS * EDGE_F], f32, kind="ExternalInput")
    wrese_d = nc.dram_tensor("wrese", [IN_F + 1, 128], f32, kind="ExternalInput")
    ident_d = nc.dram_tensor("ident", [128, 128], f32, kind="ExternalInput")
    out_d = nc.dram_tensor("out", [WNODES, 128], f32, kind="ExternalOutput")

    with tile.TileContext(nc) as tc, ExitStack() as ctx:
        const = ctx.enter_context(tc.tile_pool(name="const", bufs=1))
        dramp = ctx.enter_context(tc.tile_pool(name="dram", bufs=1, space="DRAM"))
        xp_t = dramp.tile([N // 2, 256], f32)

        ident = const.tile([128, 128], f32)
        nc.sync.dma_start(ident[:], ident_d.ap())
        wlint = const.tile([IN_F, 128], f32)
        nc.sync.dma_start(wlint[:], wlin_d.ap())
        dext_t = const.tile([IN_F + 1, HEADS], f32)
        nc.sync.dma_start(dext_t[:], dext_d.ap())
        wsrep_t = const.tile([128, 128], f32)
        nc.sync.dma_start(wsrep_t[:], wsrep_d.ap())
        crep_t = const.tile([128, HEADS * EDGE_F], f32)
        nc.sync.dma_start(crep_t[:], crep_d.ap())
        wrese_t = const.tile([IN_F + 1, 128], f32)
        nc.sync.dma_start(wrese_t[:], wrese_d.ap())
        xTown = const.tile([IN_F + 1, WNODES], f32)
        nc.sync.dma_start(xTown[:], xT_d.ap()[:, 0:WNODES])
        selL = const.tile([128, SUMT], f32)
        nc.sync.dma_start(selL[:], sel_d.ap()[0])
        selH = const.tile([128, SUMT], f32)
        nc.sync.dma_start(selH[:], sel_d.ap()[1])
        idxall = const.tile([128, IDXCOLS], i16)
        nc.sync.dma_start(idxall[:], idx_d.ap())
        atb = const.tile([128, NW * HEADS], f32)

        xp_rows = xp_t[:].rearrange("n (a f) -> (n a) f", a=2)  # [50000, 128] view

        # ---- pass-0: xp table + per-window a_t/bias columns ----
        with tc.tile_pool(name="p0", bufs=4) as p0, \
             tc.tile_pool(name="p0ps", bufs=4, space="PSUM") as p0ps:
            nblk = (N + 127) // 128
            for b in range(nblk):
                nb = min(128, N - b * 128)
                xTb = p0.tile([IN_F, 128], f32, tag="xTb")
                nc.sync.dma_start(xTb[:, :nb], xT_d.ap()[0:IN_F, b * 128: b * 128 + nb])
                ps = p0ps.tile([128, 128], f32, tag="ps")
                nc.tensor.matmul(ps[:nb, :], xTb[:, :nb], wlint[:], start=True, stop=True)
                sb = p0.tile([128, 128], f32, tag="sb")
                nc.scalar.copy(sb[:nb, :], ps[:nb, :])
                nc.sync.dma_start(xp_rows[b * 128: b * 128 + nb, :], sb[:nb, :])
            for w in range(NW):
                ps2 = p0ps.tile([128, HEADS], f32, tag="ps2")
                nc.tensor.matmul(ps2[:], xTown[:, w * 128:(w + 1) * 128], dext_t[:],
                                 start=True, stop=True)
                nc.scalar.copy(atb[:, w * HEADS:(w + 1) * HEADS], ps2[:])

        # ---- main loop ----
        with tc.tile_pool(name="xsp", bufs=3) as xsp, \
             tc.tile_pool(name="eap", bufs=3) as eap, \
             tc.tile_pool(name="scr", bufs=2) as scr, \
             tc.tile_pool(name="sml", bufs=3) as sml, \
             tc.tile_pool(name="rhsp", bufs=2) as rhsp, \
             tc.tile_pool(name="outp", bufs=3) as outp, \
             tc.tile_pool(name="mps", bufs=2, space="PSUM") as mps:

            qrr = 0
            wchunks = {}
            for ch in chunks:
                wchunks.setdefault(ch[0], []).append(ch)

            for w in range(NW):
                tw = int(T_w[w])
                num_ps = mps.tile([128, 132], f32, tag="num")
                res_ps = mps.tile([128, 128], f32, tag="res")
                nc.tensor.matmul(res_ps[:], xTown[:, w * 128:(w + 1) * 128],
                                 wrese_t[:], start=True, stop=True)
                nmm = 2 * tw
                imm = 0
                for (_, t0, t1, ic0) in wchunks[w]:
                    tcn = t1 - t0
                    nidx = tcn * 128
                    scol = int(TOFF[w]) + t0

                    xs = xsp.tile([128, TC_TILES, 256], f32, tag="xs")
                    nc.gpsimd.dma_gather(
                        xs[:, :tcn, :], xp_t[:], idxall[:, ic0: ic0 + tcn * 8],
                        nidx, nidx, 256, single_packet=False, queue_num=qrr % 4)
                    qrr += 1

                    eat = eap.tile([128, TC_TILES * EDGE_F], f32, tag="eat")
                    nc.sync.dma_start(eat[:, :tcn * EDGE_F],
                                      ea_d.ap()[:, scol * EDGE_F: (scol + tcn) * EDGE_F])

                    xs_lo = xs[:, :tcn, 0:128]
                    xs_hi = xs[:, :tcn, 128:256]
                    wsb = wsrep_t[:].rearrange("p (a f) -> p a f", a=1) \
                                    .broadcast_to([128, tcn, 128])

                    prod = scr.tile([128, TC_TILES * 128], f32, tag="prod")
                    prod_v = prod[:, :tcn * 128].rearrange("p (t f) -> p t f", t=tcn)
                    prod_g = prod[:, :tcn * 128].rearrange("p (g x) -> p g x", x=OUT_F)
                    zs_lo = sml.tile([128, TC_TILES * HEADS], f32, tag="zslo")
                    zs_hi = sml.tile([128, TC_TILES * HEADS], f32, tag="zshi")
                    nc.vector.tensor_tensor(prod_v, xs_lo, wsb, op=AL.mult)
                    nc.vector.tensor_reduce(zs_lo[:, :tcn * HEADS], prod_g,
                                            axis=mybir.AxisListType.X, op=AL.add)
                    nc.vector.tensor_tensor(prod_v, xs_hi, wsb, op=AL.mult)
                    nc.vector.tensor_reduce(zs_hi[:, :tcn * HEADS], prod_g,
                                            axis=mybir.AxisListType.X, op=AL.add)

                    prode = scr.tile([128, TC_TILES * HEADS * EDGE_F], f32, tag="prode")
                    ea_bc = eat[:, :tcn * EDGE_F] \
                        .rearrange("p (t k) -> p t k", t=tcn) \
                        .rearrange("p t (a k) -> p t a k", a=1) \
                        .broadcast_to([128, tcn, HEADS, EDGE_F])
                    crep_bc = crep_t[:].rearrange("p (a f) -> p a f", a=1) \
                        .broadcast_to([128, tcn, HEADS * EDGE_F]) \
                        .rearrange("p t (h k) -> p t h k", h=HEADS)
                    prode_v = prode[:, :tcn * HEADS * EDGE_F] \
                        .rearrange("p (t h k) -> p t h k", t=tcn, h=HEADS)
                    prode_g = prode[:, :tcn * HEADS * EDGE_F] \
                        .rearrange("p (g x) -> p g x", x=EDGE_F)
                    ze = sml.tile([128, TC_TILES * HEADS], f32, tag="ze")
                    nc.vector.tensor_tensor(prode_v, ea_bc, crep_bc, op=AL.mult)
                    nc.vector.tensor_reduce(ze[:, :tcn * HEADS], prode_g,
                                            axis=mybir.AxisListType.X, op=AL.add)

                    selLb = selL[:, scol: scol + tcn] \
                        .rearrange("p (t a) -> p t a", a=1).broadcast_to([128, tcn, HEADS])
                    selHb = selH[:, scol: scol + tcn] \
                        .rearrange("p (t a) -> p t a", a=1).broadcast_to([128, tcn, HEADS])
                    atbb = atb[:, w * HEADS:(w + 1) * HEADS] \
                        .rearrange("p (a h) -> p a h", a=1).broadcast_to([128, tcn, HEADS])

                    nh = tcn * HEADS
                    u = sml.tile([128, TC_TILES * HEADS], f32, tag="u")
                    u_v = u[:, :nh].rearrange("p (t h) -> p t h", t=tcn)
                    zs_lo_v = zs_lo[:, :nh].rearrange("p (t h) -> p t h", t=tcn)
                    zs_hi_v = zs_hi[:, :nh].rearrange("p (t h) -> p t h", t=tcn)
                    ze_v = ze[:, :nh].rearrange("p (t h) -> p t h", t=tcn)
                    # u = zs_lo*selL + zs_hi*selH + ze + atb
                    t1t = sml.tile([128, TC_TILES * HEADS], f32, tag="t1")
                    t1v = t1t[:, :nh].rearrange("p (t h) -> p t h", t=tcn)
                    nc.vector.tensor_tensor(t1v, zs_lo_v, selLb, op=AL.mult)
                    t2t = sml.tile([128, TC_TILES * HEADS], f32, tag="t2")
                    t2v = t2t[:, :nh].rearrange("p (t h) -> p t h", t=tcn)
                    nc.vector.tensor_tensor(t2v, zs_hi_v, selHb, op=AL.mult)
                    nc.vector.tensor_tensor(u_v, t1v, t2v, op=AL.add)
                    nc.vector.tensor_tensor(u_v, u_v, ze_v, op=AL.add)
                    nc.vector.tensor_tensor(u_v, u_v, atbb, op=AL.add)
                    # leaky relu: lr = max(u, 0.2*u); then exp
                    lr = sml.tile([128, TC_TILES * HEADS], f32, tag="lr")
                    nc.vector.scalar_tensor_tensor(lr[:, :nh], u[:, :nh], NEG_SLOPE,
                                                   u[:, :nh], op0=AL.mult, op1=AL.max)
                    ev = sml.tile([128, TC_TILES * HEADS], f32, tag="ev")
                    nc.scalar.activation(ev[:, :nh], lr[:, :nh],
                                         mybir.ActivationFunctionType.Exp)
                    ev_v = ev[:, :nh].rearrange("p (t h) -> p t h", t=tcn)

                    rhs = rhsp.tile([128, TC_TILES, 2, 132], f32, tag="rhs")
                    evlo = rhs[:, :tcn, 0, 128:132]
                    evhi = rhs[:, :tcn, 1, 128:132]
                    nc.vector.tensor_tensor(evlo, ev_v, selLb, op=AL.mult)
                    nc.vector.tensor_tensor(evhi, ev_v, selHb, op=AL.mult)
                    evlo_bc = evlo.rearrange("p t (h a) -> p t h a", a=1) \
                                  .broadcast_to([128, tcn, HEADS, OUT_F])
                    evhi_bc = evhi.rearrange("p t (h a) -> p t h a", a=1) \
                                  .broadcast_to([128, tcn, HEADS, OUT_F])
                    msg_lo = rhs[:, :tcn, 0, 0:128].rearrange("p t (h f) -> p t h f", h=HEADS)
                    msg_hi = rhs[:, :tcn, 1, 0:128].rearrange("p t (h f) -> p t h f", h=HEADS)
                    xs_lo4 = xs_lo.rearrange("p t (h f) -> p t h f", h=HEADS)
                    xs_hi4 = xs_hi.rearrange("p t (h f) -> p t h f", h=HEADS)
                    nc.vector.tensor_tensor(msg_lo, xs_lo4, evlo_bc, op=AL.mult)
                    nc.vector.tensor_tensor(msg_hi, xs_hi4, evhi_bc, op=AL.mult)

                    for t in range(tcn):
                        for h2 in range(2):
                            nc.tensor.matmul(num_ps[:], ident[:], rhs[:, t, h2, :],
                                             start=(imm == 0), stop=(imm == nmm - 1))
                            imm += 1

                # ---- window close ----
                dn = outp.tile([128, HEADS], f32, tag="dn")
                nc.vector.tensor_scalar_max(dn[:], num_ps[:, 128:132], 1e-30)
                rec = outp.tile([128, HEADS], f32, tag="rec")
                nc.vector.reciprocal(rec[:], dn[:])
                outw = outp.tile([128, 128], f32, tag="outw")
                outw_v = outw[:].rearrange("p (h f) -> p h f", h=HEADS)
                num_v = num_ps[:, 0:128].rearrange("p (h f) -> p h f", h=HEADS)
                rec_bc = rec[:].rearrange("p (h a) -> p h a", a=1) \
                               .broadcast_to([128, HEADS, OUT_F])
                nc.vector.tensor_tensor(outw_v, num_v, rec_bc, op=AL.mult)
                out2 = outp.tile([128, 128], f32, tag="out2")
                nc.vector.tensor_tensor(out2[:], outw[:], res_ps[:], op=AL.add)
                nc.sync.dma_start(out_d.ap()[w * 128:(w + 1) * 128, :], out2[:])

    nc.compile()
    return nc


def kernel(**inputs):
    from concourse.bass_utils import run_bass_kernel_spmd

    args = {k: np.asarray(v) for k, v in inputs.items()}
    common, per_core = _host_preprocess(
        args["x"], args["edge_index"], args["edge_attr"], args["W_lin"],
        args["w_s"], args["b_s"], args["w_t"], args["b_t"], args["W_edge"],
        args["w_e"], args["b_e"], args["W_res"], args["bias"])

    nc = _build_program(common)

    in_maps = []
    for c in range(NCORES):
        pc = per_core[c]
        in_maps.append({
            "xT": pc["xT"], "idx16": pc["idx16"], "sel": pc["sel"], "ea": pc["ea"],
            "wlinT": common["wlinT"], "dext": common["dext"], "wsrep": common["wsrep"],
            "crep": common["crep"], "wrese": common["wrese"], "ident": common["ident"],
        })

    res = run_bass_kernel_spmd(nc, in_maps, list(range(NCORES)),
                               trace=bool(os.environ.get("GAT_TRACE")))
    if os.environ.get("GAT_TRACE"):
        print(f"HW exec time: {res.exec_time_ns} ns")

    out = np.empty((N, HEADS * OUT_F), np.float32)
    for c in range(NCORES):
        out[per_core[c]["perm_owned"]] = res.results[c]["out"][:NODES_PC]
    return out


# revision 6
# speedup vs baseline: 1.1744x; 1.1744x over previous
"""GAT layer (gnn_message_passing) on 8 trn2 NeuronCores.

Strategy (dst-sharded, no collectives):
- Each core owns a contiguous 1/8 slice of target nodes; host buckets edges by
  dst core. Within a core, owned nodes are sorted by in-degree (descending) and
  grouped into 128-node windows; node -> SBUF partition, its in-edges occupy
  "slot columns" t=0..deg-1 of that partition (degree sorting makes the
  per-window column count ~= mean degree, tiny padding).
- Per edge slot, a 1280B row [xp[2j] | xp[2j+1] | a_s[2j] | a_s[2j+1] | pad]
  is fetched with SWDGE dma_gather (idx = perm_pos(src)>>1 fits int16; the
  pair covers all 50000 nodes). Table built on device in pass-0:
  xp = x @ W_lin.T, a_s = x @ fold(W_lin, w_s). Parity + slot-validity are
  folded into host sel_lo/sel_hi masks.
- Attention logits: a_e from slotted edge_attr (DVE grouped reduce with a
  replicated folded C), a_t + all scalar biases from pass-0 (x @ D_ext) as a
  per-node column. leaky-relu on DVE (scalar_tensor_tensor), exp on ACT.
  Softmax max-subtraction dropped: logits are O(1), softmax shift-invariant.
- msg = expv * xs into an rhs buffer (expv appended as 4 extra cols); window
  numerator+denominator = ONE DVE tensor_reduce(axis=XY) over the slot dims.
  Residual x @ W_res.T + bias via ones-row-extended matmul (PE, PSUM).
  out = num/denom + res.
"""
import os
import sys
from contextlib import ExitStack

sys.path.insert(0, "/opt/trn_rl_repo")

import numpy as np

N, E = 50000, 1600000
IN_F, EDGE_F, HEADS, OUT_F = 64, 16, 4, 32
NEG_SLOPE = 0.2
NCORES = 8
NODES_PC = N // NCORES            # 6250
NW = (NODES_PC + 127) // 128      # 49 windows/core
WNODES = NW * 128                 # 6272 (last window partially real)
TC_TILES = 14                     # gather-chunk size in 128-slot tiles
ROWF = 320                        # gather-table row: 256 xp-pair + 8 a_s + 56 pad


def _host_preprocess(x, edge_index, edge_attr, W_lin, w_s, b_s, w_t, b_t,
                     W_edge, w_e, b_e, W_res, bias):
    """Pure index/layout work + weight folding. Returns (common, per_core)."""
    src = edge_index[0].astype(np.int64)
    dst = edge_index[1].astype(np.int64)
    deg = np.bincount(dst, minlength=N)

    # ---- weight folding (weights only; standard operator fusion) ----
    wlinT = np.ascontiguousarray(W_lin.T)                      # [64, 128]
    C = (W_edge.reshape(HEADS, OUT_F, EDGE_F) * w_e[None, :, None]).sum(1)  # [4,16]
    crep = np.tile(C.reshape(-1)[None, :], (128, 1)).astype(np.float32)    # [128,64]
    D = (W_lin.reshape(HEADS, OUT_F, IN_F) * w_t[None, :, None]).sum(1).T  # [64,4]
    b_total = float(b_s) + float(b_t) + float(b_e)
    dext = np.vstack([D, np.full((1, HEADS), b_total, np.float32)]).astype(np.float32)
    Dws = (W_lin.reshape(HEADS, OUT_F, IN_F) * w_s[None, :, None]).sum(1).T  # [64,4]
    dws = Dws.astype(np.float32)
    wrese = np.vstack([W_res.T, bias[None, :]]).astype(np.float32)         # [65,128]

    # ---- per-core schedules (common T_w across cores) ----
    cores = []
    for c in range(NCORES):
        lo = c * NODES_PC
        owned = np.arange(lo, lo + NODES_PC)
        dc = deg[owned]
        order = np.argsort(-dc, kind="stable")
        perm_owned = owned[order]
        degs_sorted = dc[order]
        tw = np.maximum(degs_sorted[::128][:NW], 1).astype(np.int64)
        cores.append(dict(perm_owned=perm_owned, tw=tw))

    T_w = np.max(np.stack([cc["tw"] for cc in cores]), axis=0)  # [NW]
    TOFF = np.concatenate([[0], np.cumsum(T_w)])                # slot col offsets
    SUMT = int(TOFF[-1])

    chunks = []           # (w, t0, t1, icol0)
    icol = 0
    for w in range(NW):
        t = 0
        while t < T_w[w]:
            t1 = min(t + TC_TILES, int(T_w[w]))
            chunks.append((w, t, t1, icol))
            icol += (t1 - t) * 8
            t += t1 - t
    IDXCOLS = icol

    per_core = []
    for c in range(NCORES):
        cc = cores[c]
        perm_owned = cc["perm_owned"]
        rest = np.setdiff1d(np.arange(N), perm_owned, assume_unique=True)
        perm = np.concatenate([perm_owned, rest])
        perm_pos = np.empty(N, np.int64)
        perm_pos[perm] = np.arange(N)

        emask = (dst >= c * NODES_PC) & (dst < (c + 1) * NODES_PC)
        e_ids = np.nonzero(emask)[0]
        d_loc = perm_pos[dst[e_ids]]                 # 0..6249
        eorder = np.argsort(d_loc, kind="stable")
        e_s = e_ids[eorder]
        ds = d_loc[eorder]
        starts = np.searchsorted(ds, np.arange(NODES_PC))
        t_of = np.arange(len(ds)) - starts[ds]
        w_of = ds // 128
        p_of = ds % 128
        col = TOFF[w_of] + t_of

        src_rel = perm_pos[src[e_s]]
        par = (src_rel & 1).astype(np.float32)

        idx_slot = np.zeros((128, SUMT), np.int16)
        sel = np.zeros((2, 128, SUMT), np.float32)
        ea_slot = np.zeros((128, SUMT, EDGE_F), np.float32)
        idx_slot[p_of, col] = (src_rel >> 1).astype(np.int16)
        sel[0, p_of, col] = 1.0 - par
        sel[1, p_of, col] = par
        ea_slot[p_of, col] = edge_attr[e_s]

        idx16 = np.zeros((128, IDXCOLS), np.int16)
        for (w, t0, t1, ic0) in chunks:
            ncol = (t1 - t0) * 8
            flat = idx_slot[:, TOFF[w] + t0: TOFF[w] + t1].T.reshape(-1)
            wrapped = flat.reshape(-1, 16).T
            idx16[:, ic0: ic0 + ncol] = np.tile(wrapped, (8, 1))

        xT_ext = np.empty((IN_F + 1, N), np.float32)
        xT_ext[:IN_F] = x[perm].T
        xT_ext[IN_F] = 1.0

        per_core.append(dict(
            xT=xT_ext,
            idx16=idx16,
            sel=sel,
            ea=ea_slot.reshape(128, SUMT * EDGE_F),
            perm_owned=perm_owned,
        ))

    common = dict(T_w=T_w, TOFF=TOFF, SUMT=SUMT, chunks=chunks, IDXCOLS=IDXCOLS,
                  wlinT=wlinT.astype(np.float32), dext=dext, dws=dws,
                  crep=crep, wrese=wrese)
    return common, per_core


def _build_program(common):
    import concourse.bass as bass
    import concourse.tile as tile
    from concourse import bacc, mybir

    f32 = mybir.dt.float32
    i16 = mybir.dt.int16
    AL = mybir.AluOpType
    AX = mybir.AxisListType
    SUMT, IDXCOLS = common["SUMT"], common["IDXCOLS"]
    T_w, TOFF, chunks = common["T_w"], common["TOFF"], common["chunks"]

    nc = bacc.Bacc("TRN2", target_bir_lowering=False, debug=False,
                   num_devices=NCORES, num_swdge_queues=4)

    xT_d = nc.dram_tensor("xT", [IN_F + 1, N], f32, kind="ExternalInput")
    idx_d = nc.dram_tensor("idx16", [128, IDXCOLS], i16, kind="ExternalInput")
    sel_d = nc.dram_tensor("sel", [2, 128, SUMT], f32, kind="ExternalInput")
    ea_d = nc.dram_tensor("ea", [128, SUMT * EDGE_F], f32, kind="ExternalInput")
    wlin_d = nc.dram_tensor("wlinT", [IN_F, 128], f32, kind="ExternalInput")
    dext_d = nc.dram_tensor("dext", [IN_F + 1, HEADS], f32, kind="ExternalInput")
    dws_d = nc.dram_tensor("dws", [IN_F, HEADS], f32, kind="ExternalInput")
    crep_d = nc.dram_tensor("crep", [128, HEADS * EDGE_F], f32, kind="ExternalInput")
    wrese_d = nc.dram_tensor("wrese", [IN_F + 1, 128], f32, kind="ExternalInput")
    out_d = nc.dram_tensor("out", [WNODES, 128], f32, kind="ExternalOutput")

    with tile.TileContext(nc) as tc, ExitStack() as ctx:
        const = ctx.enter_context(tc.tile_pool(name="const", bufs=1))
        dramp = ctx.enter_context(tc.tile_pool(name="dram", bufs=1, space="DRAM"))
        xp_t = dramp.tile([N // 2, ROWF], f32)

        wlint = const.tile([IN_F, 128], f32)
        nc.sync.dma_start(wlint[:], wlin_d.ap())
        dext_t = const.tile([IN_F + 1, HEADS], f32)
        nc.sync.dma_start(dext_t[:], dext_d.ap())
        dws_t = const.tile([IN_F, HEADS], f32)
        nc.sync.dma_start(dws_t[:], dws_d.ap())
        crep_t = const.tile([128, HEADS * EDGE_F], f32)
        nc.sync.dma_start(crep_t[:], crep_d.ap())
        wrese_t = const.tile([IN_F + 1, 128], f32)
        nc.sync.dma_start(wrese_t[:], wrese_d.ap())
        xTown = const.tile([IN_F + 1, WNODES], f32)
        nc.sync.dma_start(xTown[:], xT_d.ap()[:, 0:WNODES])
        selL = const.tile([128, SUMT], f32)
        nc.sync.dma_start(selL[:], sel_d.ap()[0])
        selH = const.tile([128, SUMT], f32)
        nc.sync.dma_start(selH[:], sel_d.ap()[1])
        idxall = const.tile([128, IDXCOLS], i16)
        nc.sync.dma_start(idxall[:], idx_d.ap())
        atb = const.tile([128, NW * HEADS], f32)

        # ---- pass-0: gather table ([25000, 320] pair rows) + a_t columns ----
        NBLK = (N + 127) // 128          # 391 node blocks of 128
        GB = 8                           # blocks per batched table write
        SLABW = 12544                    # 98 blocks per slab (slab-aligned groups)
        with tc.tile_pool(name="p0slab", bufs=2) as slabp, \
             tc.tile_pool(name="p0", bufs=3) as p0, \
             tc.tile_pool(name="p0ps", bufs=4, space="PSUM") as p0ps:
            xp_flat = xp_t[:]            # [25000, 320]
            nslab = (N + SLABW - 1) // SLABW
            for sl in range(nslab):
                c0 = sl * SLABW
                cw = min(SLABW, N - c0)
                slab = slabp.tile([IN_F, SLABW], f32, tag="slab")
                nc.sync.dma_start(slab[:, :cw], xT_d.ap()[0:IN_F, c0:c0 + cw])
                b0 = c0 // 128
                bn = (cw + 127) // 128
                for bg in range(b0, b0 + bn, GB):
                    gn = min(GB, b0 + bn - bg)
                    stage = p0.tile([128, GB * 132], f32, tag="stage")
                    for k in range(gn):
                        b = bg + k
                        nb = min(128, N - b * 128)
                        lo = b * 128 - c0
                        if nb < 128:
                            nc.vector.memset(stage[:, k * 132:(k + 1) * 132], 0.0)
                        ps = p0ps.tile([128, 128], f32, tag="ps")
                        nc.tensor.matmul(ps[:nb, :], slab[:, lo:lo + nb],
                                         wlint[:], start=True, stop=True)
                        nc.scalar.copy(stage[:nb, k * 132:k * 132 + 128], ps[:nb, :])
                        ps2 = p0ps.tile([128, HEADS], f32, tag="ps2")
                        nc.tensor.matmul(ps2[:nb, :], slab[:, lo:lo + nb],
                                         dws_t[:], start=True, stop=True)
                        nc.scalar.copy(stage[:nb, k * 132 + 128:(k + 1) * 132],
                                       ps2[:nb, :])
                    gfull = gn
                    if bg + gn == NBLK and N % 128 != 0:
                        gfull = gn - 1
                    for par in range(2):
                        src = stage[:].rearrange("(r a) c -> a r c", a=2)[par] \
                                      .rearrange("r (k c) -> r k c", c=132)
                        if gfull > 0:
                            dst_xp = xp_flat[64 * bg: 64 * (bg + gfull),
                                             128 * par: 128 * par + 128] \
                                .rearrange("(k r) f -> r k f", k=gfull)
                            nc.sync.dma_start(dst_xp, src[:, :gfull, 0:128])
                            dst_as = xp_flat[64 * bg: 64 * (bg + gfull),
                                             256 + HEADS * par: 256 + HEADS * (par + 1)] \
                                .rearrange("(k r) h -> r k h", k=gfull)
                            nc.sync.dma_start(dst_as, src[:, :gfull, 128:132])
                        if gfull < gn:
                            b = bg + gfull
                            rows = (N - b * 128) // 2     # pair rows in partial block
                            r0 = 64 * b
                            nc.sync.dma_start(
                                xp_flat[r0: r0 + rows, 128 * par: 128 * par + 128],
                                src[:rows, gfull, 0:128])
                            nc.sync.dma_start(
                                xp_flat[r0: r0 + rows,
                                        256 + HEADS * par: 256 + HEADS * (par + 1)],
                                src[:rows, gfull, 128:132])
            for w in range(NW):
                ps2 = p0ps.tile([128, HEADS], f32, tag="ps2")
                nc.tensor.matmul(ps2[:], xTown[:, w * 128:(w + 1) * 128], dext_t[:],
                                 start=True, stop=True)
                nc.scalar.copy(atb[:, w * HEADS:(w + 1) * HEADS], ps2[:])

        # ---- main loop ----
        with tc.tile_pool(name="xsp", bufs=3) as xsp, \
             tc.tile_pool(name="eap", bufs=3) as eap, \
             tc.tile_pool(name="scr", bufs=2) as scr, \
             tc.tile_pool(name="sml", bufs=3) as sml, \
             tc.tile_pool(name="rhsp", bufs=2) as rhsp, \
             tc.tile_pool(name="nap", bufs=2) as nap, \
             tc.tile_pool(name="outp", bufs=3) as outp, \
             tc.tile_pool(name="mps", bufs=2, space="PSUM") as mps:

            qrr = 0
            wchunks = {}
            for ch in chunks:
                wchunks.setdefault(ch[0], []).append(ch)

            for w in range(NW):
                res_ps = mps.tile([128, 128], f32, tag="res")
                nc.tensor.matmul(res_ps[:], xTown[:, w * 128:(w + 1) * 128],
                                 wrese_t[:], start=True, stop=True)
                num_acc = nap.tile([128, 132], f32, tag="num")
                first = True
                for (_, t0, t1, ic0) in wchunks[w]:
                    tcn = t1 - t0
                    nidx = tcn * 128
                    scol = int(TOFF[w]) + t0

                    xs = xsp.tile([128, TC_TILES, ROWF], f32, tag="xs")
                    nc.gpsimd.dma_gather(
                        xs[:, :tcn, :], xp_t[:], idxall[:, ic0: ic0 + tcn * 8],
                        nidx, nidx, ROWF, single_packet=False, queue_num=qrr % 4)
                    qrr += 1

                    eat = eap.tile([128, TC_TILES * EDGE_F], f32, tag="eat")
                    nc.sync.dma_start(eat[:, :tcn * EDGE_F],
                                      ea_d.ap()[:, scol * EDGE_F: (scol + tcn) * EDGE_F])

                    # a_e: grouped reduce of ea * C
                    prode = scr.tile([128, TC_TILES * HEADS * EDGE_F], f32, tag="prode")
                    ea_bc = eat[:, :tcn * EDGE_F] \
                        .rearrange("p (t k) -> p t k", t=tcn) \
                        .rearrange("p t (a k) -> p t a k", a=1) \
                        .broadcast_to([128, tcn, HEADS, EDGE_F])
                    crep_bc = crep_t[:].rearrange("p (a f) -> p a f", a=1) \
                        .broadcast_to([128, tcn, HEADS * EDGE_F]) \
                        .rearrange("p t (h k) -> p t h k", h=HEADS)
                    prode_v = prode[:, :tcn * HEADS * EDGE_F] \
                        .rearrange("p (t h k) -> p t h k", t=tcn, h=HEADS)
                    prode_g = prode[:, :tcn * HEADS * EDGE_F] \
                        .rearrange("p (g x) -> p g x", x=EDGE_F)
                    ze = sml.tile([128, TC_TILES * HEADS], f32, tag="ze")
                    nc.vector.tensor_tensor(prode_v, ea_bc, crep_bc, op=AL.mult)
                    nc.vector.tensor_reduce(ze[:, :tcn * HEADS], prode_g,
                                            axis=AX.X, op=AL.add)

                    selLb = selL[:, scol: scol + tcn] \
                        .rearrange("p (t a) -> p t a", a=1).broadcast_to([128, tcn, HEADS])
                    selHb = selH[:, scol: scol + tcn] \
                        .rearrange("p (t a) -> p t a", a=1).broadcast_to([128, tcn, HEADS])
                    atbb = atb[:, w * HEADS:(w + 1) * HEADS] \
                        .rearrange("p (a h) -> p a h", a=1).broadcast_to([128, tcn, HEADS])

                    nh = tcn * HEADS
                    # u = as_lo*selL + as_hi*selH + ze + atb  (a_s slices ride the rows)
                    as_lo = xs[:, :tcn, 256:256 + HEADS]
                    as_hi = xs[:, :tcn, 256 + HEADS:256 + 2 * HEADS]
                    t1t = sml.tile([128, TC_TILES * HEADS], f32, tag="t1")
                    t1v = t1t[:, :nh].rearrange("p (t h) -> p t h", t=tcn)
                    nc.vector.tensor_tensor(t1v, as_lo, selLb, op=AL.mult)
                    t2t = sml.tile([128, TC_TILES * HEADS], f32, tag="t2")
                    t2v = t2t[:, :nh].rearrange("p (t h) -> p t h", t=tcn)
                    nc.vector.tensor_tensor(t2v, as_hi, selHb, op=AL.mult)
                    u = sml.tile([128, TC_TILES * HEADS], f32, tag="u")
                    u_v = u[:, :nh].rearrange("p (t h) -> p t h", t=tcn)
                    ze_v = ze[:, :nh].rearrange("p (t h) -> p t h", t=tcn)
                    nc.vector.tensor_tensor(u_v, t1v, t2v, op=AL.add)
                    nc.vector.tensor_tensor(u_v, u_v, ze_v, op=AL.add)
                    nc.vector.tensor_tensor(u_v, u_v, atbb, op=AL.add)
                    lr = sml.tile([128, TC_TILES * HEADS], f32, tag="lr")
                    nc.vector.scalar_tensor_tensor(lr[:, :nh], u[:, :nh], NEG_SLOPE,
                                                   u[:, :nh], op0=AL.mult, op1=AL.max)
                    ev = sml.tile([128, TC_TILES * HEADS], f32, tag="ev")
                    nc.scalar.activation(ev[:, :nh], lr[:, :nh],
                                         mybir.ActivationFunctionType.Exp)
                    ev_v = ev[:, :nh].rearrange("p (t h) -> p t h", t=tcn)

                    rhs = rhsp.tile([128, TC_TILES, 2, 132], f32, tag="rhs")
                    evlo = rhs[:, :tcn, 0, 128:132]
                    evhi = rhs[:, :tcn, 1, 128:132]
                    nc.vector.tensor_tensor(evlo, ev_v, selLb, op=AL.mult)
                    nc.vector.tensor_tensor(evhi, ev_v, selHb, op=AL.mult)
                    evlo_bc = evlo.rearrange("p t (h a) -> p t h a", a=1) \
                                  .broadcast_to([128, tcn, HEADS, OUT_F])
                    evhi_bc = evhi.rearrange("p t (h a) -> p t h a", a=1) \
                                  .broadcast_to([128, tcn, HEADS, OUT_F])
                    msg_lo = rhs[:, :tcn, 0, 0:128].rearrange("p t (h f) -> p t h f", h=HEADS)
                    msg_hi = rhs[:, :tcn, 1, 0:128].rearrange("p t (h f) -> p t h f", h=HEADS)
                    xs_lo4 = xs[:, :tcn, 0:128].rearrange("p t (h f) -> p t h f", h=HEADS)
                    xs_hi4 = xs[:, :tcn, 128:256].rearrange("p t (h f) -> p t h f", h=HEADS)
                    nc.vector.tensor_tensor(msg_lo, xs_lo4, evlo_bc, op=AL.mult)
                    nc.vector.tensor_tensor(msg_hi, xs_hi4, evhi_bc, op=AL.mult)

                    # num += sum over (t, half) of rhs  -> reduce innermost 2 dims
                    red_in = rhs[:, :tcn, :, :].rearrange("p t h f -> p f t h")
                    if first:
                        nc.vector.tensor_reduce(num_acc[:], red_in, axis=AX.XY,
                                                op=AL.add)
                        first = False
                    else:
                        part = nap.tile([128, 132], f32, tag="part")
                        nc.vector.tensor_reduce(part[:], red_in, axis=AX.XY, op=AL.add)
                        nc.vector.tensor_tensor(num_acc[:], num_acc[:], part[:],
                                                op=AL.add)

                # ---- window close ----
                dn = outp.tile([128, HEADS], f32, tag="dn")
                nc.vector.tensor_scalar_max(dn[:], num_acc[:, 128:132], 1e-30)
                rec = outp.tile([128, HEADS], f32, tag="rec")
                nc.vector.reciprocal(rec[:], dn[:])
                outw = outp.tile([128, 128], f32, tag="outw")
                outw_v = outw[:].rearrange("p (h f) -> p h f", h=HEADS)
                num_v = num_acc[:, 0:128].rearrange("p (h f) -> p h f", h=HEADS)
                rec_bc = rec[:].rearrange("p (h a) -> p h a", a=1) \
                               .broadcast_to([128, HEADS, OUT_F])
                nc.vector.tensor_tensor(outw_v, num_v, rec_bc, op=AL.mult)
                out2 = outp.tile([128, 128], f32, tag="out2")
                nc.vector.tensor_tensor(out2[:], outw[:], res_ps[:], op=AL.add)
                nc.sync.dma_start(out_d.ap()[w * 128:(w + 1) * 128, :], out2[:])

    nc.compile()
    return nc


def kernel(**inputs):
    from concourse.bass_utils import run_bass_kernel_spmd

    args = {k: np.asarray(v) for k, v in inputs.items()}
    common, per_core = _host_preprocess(
        args["x"], args["edge_index"], args["edge_attr"], args["W_lin"],
        args["w_s"], args["b_s"], args["w_t"], args["b_t"], args["W_edge"],
        args["w_e"], args["b_e"], args["W_res"], args["bias"])

    nc = _build_program(common)

    in_maps = []
    for c in range(NCORES):
        pc = per_core[c]
        in_maps.append({
            "xT": pc["xT"], "idx16": pc["idx16"], "sel": pc["sel"], "ea": pc["ea"],
            "wlinT": common["wlinT"], "dext": common["dext"], "dws": common["dws"],
            "crep": common["crep"], "wrese": common["wrese"],
        })

    res = run_bass_kernel_spmd(nc, in_maps, list(range(NCORES)),
                               trace=bool(os.environ.get("GAT_TRACE")),
                               tmpdir=os.environ.get("GAT_TMPDIR"))
    if os.environ.get("GAT_TRACE"):
        print(f"HW exec time: {res.exec_time_ns} ns")

    out = np.empty((N, HEADS * OUT_F), np.float32)
    for c in range(NCORES):
        out[per_core[c]["perm_owned"]] = res.results[c]["out"][:NODES_PC]
    return out


# revision 7
# speedup vs baseline: 1.2873x; 1.0961x over previous
"""GAT layer (gnn_message_passing) on 8 trn2 NeuronCores.

Strategy (dst-sharded, no collectives):
- Each core owns a contiguous 1/8 slice of target nodes; host buckets edges by
  dst core. Within a core, owned nodes are sorted by in-degree (descending) and
  grouped into 128-node windows; node -> SBUF partition, its in-edges occupy
  "slot columns" t=0..deg-1 of that partition (degree sorting makes the
  per-window column count ~= mean degree, tiny padding).
- Per edge slot, a 1280B row [xp[2j] | xp[2j+1] | a_s[2j] | a_s[2j+1] | pad]
  is fetched with SWDGE dma_gather (idx = perm_pos(src)>>1 fits int16; the
  pair covers all 50000 nodes). Table built on device in pass-0:
  xp = x @ W_lin.T, a_s = x @ fold(W_lin, w_s). Parity + slot-validity are
  folded into host sel_lo/sel_hi masks.
- Attention logits: a_e from slotted edge_attr (DVE grouped reduce with a
  replicated folded C), a_t + all scalar biases from pass-0 (x @ D_ext) as a
  per-node column. leaky-relu on DVE (scalar_tensor_tensor), exp on ACT.
  Softmax max-subtraction dropped: logits are O(1), softmax shift-invariant.
- msg = expv * xs into an rhs buffer (expv appended as 4 extra cols); window
  numerator+denominator = ONE DVE tensor_reduce(axis=XY) over the slot dims.
  Residual x @ W_res.T + bias via ones-row-extended matmul (PE, PSUM).
  out = num/denom + res.
"""
import os
import sys
from contextlib import ExitStack

sys.path.insert(0, "/opt/trn_rl_repo")

import numpy as np

N, E = 50000, 1600000
IN_F, EDGE_F, HEADS, OUT_F = 64, 16, 4, 32
NEG_SLOPE = 0.2
NCORES = 8
NODES_PC = N // NCORES            # 6250
NW = (NODES_PC + 127) // 128      # 49 windows/core
WNODES = NW * 128                 # 6272 (last window partially real)
TC_TILES = 14                     # gather-chunk size in 128-slot tiles
ROWF = 320                        # gather-table row: 256 xp-pair + 8 a_s + 56 pad


def _host_preprocess(x, edge_index, edge_attr, W_lin, w_s, b_s, w_t, b_t,
                     W_edge, w_e, b_e, W_res, bias):
    """Pure index/layout work + weight folding. Returns (common, per_core)."""
    src = edge_index[0].astype(np.int64)
    dst = edge_index[1].astype(np.int64)
    deg = np.bincount(dst, minlength=N)

    # ---- weight folding (weights only; standard operator fusion) ----
    wlinT = np.ascontiguousarray(W_lin.T)                      # [64, 128]
    C = (W_edge.reshape(HEADS, OUT_F, EDGE_F) * w_e[None, :, None]).sum(1)  # [4,16]
    crep = np.tile(C.reshape(-1)[None, :], (128, 1)).astype(np.float32)    # [128,64]
    D = (W_lin.reshape(HEADS, OUT_F, IN_F) * w_t[None, :, None]).sum(1).T  # [64,4]
    b_total = float(b_s) + float(b_t) + float(b_e)
    dext = np.vstack([D, np.full((1, HEADS), b_total, np.float32)]).astype(np.float32)
    Dws = (W_lin.reshape(HEADS, OUT_F, IN_F) * w_s[None, :, None]).sum(1).T  # [64,4]
    dws = Dws.astype(np.float32)
    wrese = np.vstack([W_res.T, bias[None, :]]).astype(np.float32)         # [65,128]

    # ---- per-core schedules (common T_w across cores) ----
    cores = []
    for c in range(NCORES):
        lo = c * NODES_PC
        owned = np.arange(lo, lo + NODES_PC)
        dc = deg[owned]
        order = np.argsort(-dc, kind="stable")
        perm_owned = owned[order]
        degs_sorted = dc[order]
        tw = np.maximum(degs_sorted[::128][:NW], 1).astype(np.int64)
        cores.append(dict(perm_owned=perm_owned, tw=tw))

    T_w = np.max(np.stack([cc["tw"] for cc in cores]), axis=0)  # [NW]
    TOFF = np.concatenate([[0], np.cumsum(T_w)])                # slot col offsets
    SUMT = int(TOFF[-1])

    chunks = []           # (w, t0, t1, icol0)
    icol = 0
    for w in range(NW):
        t = 0
        while t < T_w[w]:
            t1 = min(t + TC_TILES, int(T_w[w]))
            chunks.append((w, t, t1, icol))
            icol += (t1 - t) * 8
            t += t1 - t
    IDXCOLS = icol

    per_core = []
    for c in range(NCORES):
        cc = cores[c]
        perm_owned = cc["perm_owned"]
        rest = np.setdiff1d(np.arange(N), perm_owned, assume_unique=True)
        perm = np.concatenate([perm_owned, rest])
        perm_pos = np.empty(N, np.int64)
        perm_pos[perm] = np.arange(N)

        emask = (dst >= c * NODES_PC) & (dst < (c + 1) * NODES_PC)
        e_ids = np.nonzero(emask)[0]
        d_loc = perm_pos[dst[e_ids]]                 # 0..6249
        eorder = np.argsort(d_loc, kind="stable")
        e_s = e_ids[eorder]
        ds = d_loc[eorder]
        starts = np.searchsorted(ds, np.arange(NODES_PC))
        t_of = np.arange(len(ds)) - starts[ds]
        w_of = ds // 128
        p_of = ds % 128
        col = TOFF[w_of] + t_of

        src_rel = perm_pos[src[e_s]]
        par = (src_rel & 1).astype(np.float32)

        idx_slot = np.zeros((128, SUMT), np.int16)
        sel = np.zeros((2, 128, SUMT), np.float32)
        ea_slot = np.zeros((128, SUMT, EDGE_F), np.float32)
        idx_slot[p_of, col] = (src_rel >> 1).astype(np.int16)
        sel[0, p_of, col] = 1.0 - par
        sel[1, p_of, col] = par
        ea_slot[p_of, col] = edge_attr[e_s]

        idx16 = np.zeros((128, IDXCOLS), np.int16)
        for (w, t0, t1, ic0) in chunks:
            ncol = (t1 - t0) * 8
            flat = idx_slot[:, TOFF[w] + t0: TOFF[w] + t1].T.reshape(-1)
            wrapped = flat.reshape(-1, 16).T
            idx16[:, ic0: ic0 + ncol] = np.tile(wrapped, (8, 1))

        xT_ext = np.empty((IN_F + 1, N), np.float32)
        xT_ext[:IN_F] = x[perm].T
        xT_ext[IN_F] = 1.0

        per_core.append(dict(
            xT=xT_ext,
            idx16=idx16,
            sel=sel,
            ea=ea_slot.reshape(128, SUMT * EDGE_F),
            perm_owned=perm_owned,
        ))

    wlind = np.concatenate([wlinT.astype(np.float32), dws], axis=1)  # [64, 132]
    common = dict(T_w=T_w, TOFF=TOFF, SUMT=SUMT, chunks=chunks, IDXCOLS=IDXCOLS,
                  wlind=wlind, dext=dext, crep=crep, wrese=wrese)
    return common, per_core


def _build_program(common):
    import concourse.bass as bass
    import concourse.tile as tile
    from concourse import bacc, mybir

    f32 = mybir.dt.float32
    i16 = mybir.dt.int16
    AL = mybir.AluOpType
    AX = mybir.AxisListType
    SUMT, IDXCOLS = common["SUMT"], common["IDXCOLS"]
    T_w, TOFF, chunks = common["T_w"], common["TOFF"], common["chunks"]

    nc = bacc.Bacc("TRN2", target_bir_lowering=False, debug=False,
                   num_devices=NCORES, num_swdge_queues=4)

    xT_d = nc.dram_tensor("xT", [IN_F + 1, N], f32, kind="ExternalInput")
    idx_d = nc.dram_tensor("idx16", [128, IDXCOLS], i16, kind="ExternalInput")
    sel_d = nc.dram_tensor("sel", [2, 128, SUMT], f32, kind="ExternalInput")
    ea_d = nc.dram_tensor("ea", [128, SUMT * EDGE_F], f32, kind="ExternalInput")
    wlin_d = nc.dram_tensor("wlind", [IN_F, 132], f32, kind="ExternalInput")
    dext_d = nc.dram_tensor("dext", [IN_F + 1, HEADS], f32, kind="ExternalInput")
    crep_d = nc.dram_tensor("crep", [128, HEADS * EDGE_F], f32, kind="ExternalInput")
    wrese_d = nc.dram_tensor("wrese", [IN_F + 1, 128], f32, kind="ExternalInput")
    out_d = nc.dram_tensor("out", [WNODES, 128], f32, kind="ExternalOutput")

    with tile.TileContext(nc) as tc, ExitStack() as ctx:
        const = ctx.enter_context(tc.tile_pool(name="const", bufs=1))
        dramp = ctx.enter_context(tc.tile_pool(name="dram", bufs=1, space="DRAM"))
        xp_t = dramp.tile([N // 2, ROWF], f32)

        wlint = const.tile([IN_F, 132], f32)
        nc.sync.dma_start(wlint[:], wlin_d.ap())
        dext_t = const.tile([IN_F + 1, HEADS], f32)
        nc.sync.dma_start(dext_t[:], dext_d.ap())
        crep_t = const.tile([128, HEADS * EDGE_F], f32)
        nc.sync.dma_start(crep_t[:], crep_d.ap())
        wrese_t = const.tile([IN_F + 1, 128], f32)
        nc.sync.dma_start(wrese_t[:], wrese_d.ap())
        xTown = const.tile([IN_F + 1, WNODES], f32)
        nc.sync.dma_start(xTown[:], xT_d.ap()[:, 0:WNODES])
        selL = const.tile([128, SUMT], f32)
        nc.sync.dma_start(selL[:], sel_d.ap()[0])
        selH = const.tile([128, SUMT], f32)
        nc.sync.dma_start(selH[:], sel_d.ap()[1])
        idxall = const.tile([128, IDXCOLS], i16)
        nc.sync.dma_start(idxall[:], idx_d.ap())
        atb = const.tile([128, NW * HEADS], f32)

        # ---- pass-0: gather table ([25000, 320] pair rows) + a_t columns ----
        NBLK = (N + 127) // 128          # 391 node blocks of 128
        GB = 8                           # blocks per batched table write
        SLABW = 12544                    # 98 blocks per slab (slab-aligned groups)
        with tc.tile_pool(name="p0slab", bufs=2) as slabp, \
             tc.tile_pool(name="p0", bufs=3) as p0, \
             tc.tile_pool(name="p0ps", bufs=4, space="PSUM") as p0ps:
            xp_flat = xp_t[:]            # [25000, 320]
            nslab = (N + SLABW - 1) // SLABW
            for sl in range(nslab):
                c0 = sl * SLABW
                cw = min(SLABW, N - c0)
                slab = slabp.tile([IN_F, SLABW], f32, tag="slab")
                nc.sync.dma_start(slab[:, :cw], xT_d.ap()[0:IN_F, c0:c0 + cw])
                b0 = c0 // 128
                bn = (cw + 127) // 128
                for bg in range(b0, b0 + bn, GB):
                    gn = min(GB, b0 + bn - bg)
                    stage = p0.tile([128, GB * 132], f32, tag="stage")
                    for k in range(gn):
                        b = bg + k
                        nb = min(128, N - b * 128)
                        lo = b * 128 - c0
                        if nb < 128:
                            nc.vector.memset(stage[:, k * 132:(k + 1) * 132], 0.0)
                        ps = p0ps.tile([128, 132], f32, tag="ps")
                        nc.tensor.matmul(ps[:nb, :], slab[:, lo:lo + nb],
                                         wlint[:], start=True, stop=True)
                        nc.scalar.copy(stage[:nb, k * 132:(k + 1) * 132], ps[:nb, :])
                    gfull = gn
                    if bg + gn == NBLK and N % 128 != 0:
                        gfull = gn - 1
                    for par in range(2):
                        src = stage[:].rearrange("(r a) c -> a r c", a=2)[par] \
                                      .rearrange("r (k c) -> r k c", c=132)
                        if gfull > 0:
                            dst_xp = xp_flat[64 * bg: 64 * (bg + gfull),
                                             128 * par: 128 * par + 128] \
                                .rearrange("(k r) f -> r k f", k=gfull)
                            nc.sync.dma_start(dst_xp, src[:, :gfull, 0:128])
                            dst_as = xp_flat[64 * bg: 64 * (bg + gfull),
                                             256 + HEADS * par: 256 + HEADS * (par + 1)] \
                                .rearrange("(k r) h -> r k h", k=gfull)
                            nc.sync.dma_start(dst_as, src[:, :gfull, 128:132])
                        if gfull < gn:
                            b = bg + gfull
                            rows = (N - b * 128) // 2     # pair rows in partial block
                            r0 = 64 * b
                            nc.sync.dma_start(
                                xp_flat[r0: r0 + rows, 128 * par: 128 * par + 128],
                                src[:rows, gfull, 0:128])
                            nc.sync.dma_start(
                                xp_flat[r0: r0 + rows,
                                        256 + HEADS * par: 256 + HEADS * (par + 1)],
                                src[:rows, gfull, 128:132])
            for w in range(NW):
                ps2 = p0ps.tile([128, HEADS], f32, tag="ps2")
                nc.tensor.matmul(ps2[:], xTown[:, w * 128:(w + 1) * 128], dext_t[:],
                                 start=True, stop=True)
                nc.scalar.copy(atb[:, w * HEADS:(w + 1) * HEADS], ps2[:])

        # ---- main loop ----
        with tc.tile_pool(name="xsp", bufs=3) as xsp, \
             tc.tile_pool(name="eap", bufs=3) as eap, \
             tc.tile_pool(name="scr", bufs=2) as scr, \
             tc.tile_pool(name="sml", bufs=3) as sml, \
             tc.tile_pool(name="rhsp", bufs=2) as rhsp, \
             tc.tile_pool(name="nap", bufs=2) as nap, \
             tc.tile_pool(name="outp", bufs=3) as outp, \
             tc.tile_pool(name="mps", bufs=2, space="PSUM") as mps:

            qrr = 0
            wchunks = {}
            for ch in chunks:
                wchunks.setdefault(ch[0], []).append(ch)

            for w in range(NW):
                res_ps = mps.tile([128, 128], f32, tag="res")
                nc.tensor.matmul(res_ps[:], xTown[:, w * 128:(w + 1) * 128],
                                 wrese_t[:], start=True, stop=True)
                num_acc = nap.tile([128, 132], f32, tag="num")
                first = True
                for (_, t0, t1, ic0) in wchunks[w]:
                    tcn = t1 - t0
                    nidx = tcn * 128
                    scol = int(TOFF[w]) + t0

                    xs = xsp.tile([128, TC_TILES, ROWF], f32, tag="xs")
                    nc.gpsimd.dma_gather(
                        xs[:, :tcn, :], xp_t[:], idxall[:, ic0: ic0 + tcn * 8],
                        nidx, nidx, ROWF, single_packet=False, queue_num=qrr % 4)
                    qrr += 1

                    eat = eap.tile([128, TC_TILES * EDGE_F], f32, tag="eat")
                    nc.sync.dma_start(eat[:, :tcn * EDGE_F],
                                      ea_d.ap()[:, scol * EDGE_F: (scol + tcn) * EDGE_F])

                    # a_e: grouped reduce of ea * C
                    prode = scr.tile([128, TC_TILES * HEADS * EDGE_F], f32, tag="prode")
                    ea_bc = eat[:, :tcn * EDGE_F] \
                        .rearrange("p (t k) -> p t k", t=tcn) \
                        .rearrange("p t (a k) -> p t a k", a=1) \
                        .broadcast_to([128, tcn, HEADS, EDGE_F])
                    crep_bc = crep_t[:].rearrange("p (a f) -> p a f", a=1) \
                        .broadcast_to([128, tcn, HEADS * EDGE_F]) \
                        .rearrange("p t (h k) -> p t h k", h=HEADS)
                    prode_v = prode[:, :tcn * HEADS * EDGE_F] \
                        .rearrange("p (t h k) -> p t h k", t=tcn, h=HEADS)
                    prode_g = prode[:, :tcn * HEADS * EDGE_F] \
                        .rearrange("p (g x) -> p g x", x=EDGE_F)
                    ze = sml.tile([128, TC_TILES * HEADS], f32, tag="ze")
                    nc.vector.tensor_tensor(prode_v, ea_bc, crep_bc, op=AL.mult)
                    nc.vector.tensor_reduce(ze[:, :tcn * HEADS], prode_g,
                                            axis=AX.X, op=AL.add)

                    selLb = selL[:, scol: scol + tcn] \
                        .rearrange("p (t a) -> p t a", a=1).broadcast_to([128, tcn, HEADS])
                    selHb = selH[:, scol: scol + tcn] \
                        .rearrange("p (t a) -> p t a", a=1).broadcast_to([128, tcn, HEADS])
                    atbb = atb[:, w * HEADS:(w + 1) * HEADS] \
                        .rearrange("p (a h) -> p a h", a=1).broadcast_to([128, tcn, HEADS])

                    nh = tcn * HEADS
                    # u = as_lo*selL + as_hi*selH + ze + atb  (a_s slices ride the rows)
                    as_lo = xs[:, :tcn, 256:256 + HEADS]
                    as_hi = xs[:, :tcn, 256 + HEADS:256 + 2 * HEADS]
                    t1t = sml.tile([128, TC_TILES * HEADS], f32, tag="t1")
                    t1v = t1t[:, :nh].rearrange("p (t h) -> p t h", t=tcn)
                    nc.vector.tensor_tensor(t1v, as_lo, selLb, op=AL.mult)
                    t2t = sml.tile([128, TC_TILES * HEADS], f32, tag="t2")
                    t2v = t2t[:, :nh].rearrange("p (t h) -> p t h", t=tcn)
                    nc.vector.tensor_tensor(t2v, as_hi, selHb, op=AL.mult)
                    u = sml.tile([128, TC_TILES * HEADS], f32, tag="u")
                    u_v = u[:, :nh].rearrange("p (t h) -> p t h", t=tcn)
                    ze_v = ze[:, :nh].rearrange("p (t h) -> p t h", t=tcn)
                    nc.vector.tensor_tensor(u_v, t1v, t2v, op=AL.add)
                    nc.vector.tensor_tensor(u_v, u_v, ze_v, op=AL.add)
                    nc.vector.tensor_tensor(u_v, u_v, atbb, op=AL.add)
                    lr = sml.tile([128, TC_TILES * HEADS], f32, tag="lr")
                    nc.vector.scalar_tensor_tensor(lr[:, :nh], u[:, :nh], NEG_SLOPE,
                                                   u[:, :nh], op0=AL.mult, op1=AL.max)
                    ev = sml.tile([128, TC_TILES * HEADS], f32, tag="ev")
                    nc.scalar.activation(ev[:, :nh], lr[:, :nh],
                                         mybir.ActivationFunctionType.Exp)
                    ev_v = ev[:, :nh].rearrange("p (t h) -> p t h", t=tcn)

                    rhs = rhsp.tile([128, TC_TILES, 2, 132], f32, tag="rhs")
                    evlo = rhs[:, :tcn, 0, 128:132]
                    evhi = rhs[:, :tcn, 1, 128:132]
                    nc.vector.tensor_tensor(evlo, ev_v, selLb, op=AL.mult)
                    nc.vector.tensor_tensor(evhi, ev_v, selHb, op=AL.mult)
                    evlo_bc = evlo.rearrange("p t (h a) -> p t h a", a=1) \
                                  .broadcast_to([128, tcn, HEADS, OUT_F])
                    evhi_bc = evhi.rearrange("p t (h a) -> p t h a", a=1) \
                                  .broadcast_to([128, tcn, HEADS, OUT_F])
                    msg_lo = rhs[:, :tcn, 0, 0:128].rearrange("p t (h f) -> p t h f", h=HEADS)
                    msg_hi = rhs[:, :tcn, 1, 0:128].rearrange("p t (h f) -> p t h f", h=HEADS)
                    xs_lo4 = xs[:, :tcn, 0:128].rearrange("p t (h f) -> p t h f", h=HEADS)
                    xs_hi4 = xs[:, :tcn, 128:256].rearrange("p t (h f) -> p t h f", h=HEADS)
                    nc.vector.tensor_tensor(msg_lo, xs_lo4, evlo_bc, op=AL.mult)
                    nc.vector.tensor_tensor(msg_hi, xs_hi4, evhi_bc, op=AL.mult)

                    # num += sum over (t, half): contiguous pairwise fold
                    flat = rhs[:].rearrange("p t h f -> p (t h) f")   # [128, 2*TC, 132]
                    n = 2 * tcn
                    while n > 1:
                        k = n // 2
                        nc.vector.tensor_tensor(flat[:, 0:k, :], flat[:, 0:k, :],
                                                flat[:, n - k:n, :], op=AL.add)
                        n -= k
                    if first:
                        nc.vector.tensor_copy(num_acc[:], flat[:, 0, :])
                        first = False
                    else:
                        nc.vector.tensor_tensor(num_acc[:], num_acc[:], flat[:, 0, :],
                                                op=AL.add)

                # ---- window close ----
                dn = outp.tile([128, HEADS], f32, tag="dn")
                nc.vector.tensor_scalar_max(dn[:], num_acc[:, 128:132], 1e-30)
                rec = outp.tile([128, HEADS], f32, tag="rec")
                nc.vector.reciprocal(rec[:], dn[:])
                outw = outp.tile([128, 128], f32, tag="outw")
                outw_v = outw[:].rearrange("p (h f) -> p h f", h=HEADS)
                num_v = num_acc[:, 0:128].rearrange("p (h f) -> p h f", h=HEADS)
                rec_bc = rec[:].rearrange("p (h a) -> p h a", a=1) \
                               .broadcast_to([128, HEADS, OUT_F])
                nc.vector.tensor_tensor(outw_v, num_v, rec_bc, op=AL.mult)
                out2 = outp.tile([128, 128], f32, tag="out2")
                nc.vector.tensor_tensor(out2[:], outw[:], res_ps[:], op=AL.add)
                nc.sync.dma_start(out_d.ap()[w * 128:(w + 1) * 128, :], out2[:])

    nc.compile()
    return nc


def kernel(**inputs):
    from concourse.bass_utils import run_bass_kernel_spmd

    args = {k: np.asarray(v) for k, v in inputs.items()}
    common, per_core = _host_preprocess(
        args["x"], args["edge_index"], args["edge_attr"], args["W_lin"],
        args["w_s"], args["b_s"], args["w_t"], args["b_t"], args["W_edge"],
        args["w_e"], args["b_e"], args["W_res"], args["bias"])

    nc = _build_program(common)

    in_maps = []
    for c in range(NCORES):
        pc = per_core[c]
        in_maps.append({
            "xT": pc["xT"], "idx16": pc["idx16"], "sel": pc["sel"], "ea": pc["ea"],
            "wlind": common["wlind"], "dext": common["dext"],
            "crep": common["crep"], "wrese": common["wrese"],
        })

    res = run_bass_kernel_spmd(nc, in_maps, list(range(NCORES)),
                               trace=bool(os.environ.get("GAT_TRACE")),
                               tmpdir=os.environ.get("GAT_TMPDIR"))
    if os.environ.get("GAT_TRACE"):
        print(f"HW exec time: {res.exec_time_ns} ns")

    out = np.empty((N, HEADS * OUT_F), np.float32)
    for c in range(NCORES):
        out[per_core[c]["perm_owned"]] = res.results[c]["out"][:NODES_PC]
    return out


# revision 8
# speedup vs baseline: 1.2892x; 1.0015x over previous
"""GAT layer (gnn_message_passing) on 8 trn2 NeuronCores.

Strategy (dst-sharded, no collectives):
- Each core owns a contiguous 1/8 slice of target nodes; host buckets edges by
  dst core. Within a core, owned nodes are sorted by in-degree (descending) and
  grouped into 128-node windows; node -> SBUF partition, its in-edges occupy
  "slot columns" t=0..deg-1 of that partition (degree sorting makes the
  per-window column count ~= mean degree, tiny padding).
- Per edge slot, a 1280B row [xp[2j] | xp[2j+1] | a_s[2j] | a_s[2j+1] | pad]
  is fetched with SWDGE dma_gather (idx = perm_pos(src)>>1 fits int16; the
  pair covers all 50000 nodes). Table built on device in pass-0:
  xp = x @ W_lin.T, a_s = x @ fold(W_lin, w_s). Parity + slot-validity are
  folded into host sel_lo/sel_hi masks.
- Attention logits: a_e from slotted edge_attr (DVE grouped reduce with a
  replicated folded C), a_t + all scalar biases from pass-0 (x @ D_ext) as a
  per-node column. leaky-relu on DVE (scalar_tensor_tensor), exp on ACT.
  Softmax max-subtraction dropped: logits are O(1), softmax shift-invariant.
- msg = expv * xs into an rhs buffer (expv appended as 4 extra cols); window
  numerator+denominator = ONE DVE tensor_reduce(axis=XY) over the slot dims.
  Residual x @ W_res.T + bias via ones-row-extended matmul (PE, PSUM).
  out = num/denom + res.
"""
import os
import sys
from contextlib import ExitStack

sys.path.insert(0, "/opt/trn_rl_repo")

import numpy as np

N, E = 50000, 1600000
IN_F, EDGE_F, HEADS, OUT_F = 64, 16, 4, 32
NEG_SLOPE = 0.2
NCORES = 8
NODES_PC = N // NCORES            # 6250
NW = (NODES_PC + 127) // 128      # 49 windows/core
WNODES = NW * 128                 # 6272 (last window partially real)
TC_TILES = 12                     # gather-chunk size in 128-slot tiles
ROWF = 320                        # gather-table row: 256 xp-pair + 8 a_s + 56 pad


def _host_preprocess(x, edge_index, edge_attr, W_lin, w_s, b_s, w_t, b_t,
                     W_edge, w_e, b_e, W_res, bias):
    """Pure index/layout work + weight folding. Returns (common, per_core)."""
    src = edge_index[0].astype(np.int64)
    dst = edge_index[1].astype(np.int64)
    deg = np.bincount(dst, minlength=N)

    # ---- weight folding (weights only; standard operator fusion) ----
    wlinT = np.ascontiguousarray(W_lin.T)                      # [64, 128]
    C = (W_edge.reshape(HEADS, OUT_F, EDGE_F) * w_e[None, :, None]).sum(1)  # [4,16]
    crep = np.tile(C.reshape(-1)[None, :], (128, 1)).astype(np.float32)    # [128,64]
    D = (W_lin.reshape(HEADS, OUT_F, IN_F) * w_t[None, :, None]).sum(1).T  # [64,4]
    b_total = float(b_s) + float(b_t) + float(b_e)
    dext = np.vstack([D, np.full((1, HEADS), b_total, np.float32)]).astype(np.float32)
    Dws = (W_lin.reshape(HEADS, OUT_F, IN_F) * w_s[None, :, None]).sum(1).T  # [64,4]
    dws = Dws.astype(np.float32)
    wrese = np.vstack([W_res.T, bias[None, :]]).astype(np.float32)         # [65,128]

    # ---- per-core schedules (common T_w across cores) ----
    cores = []
    for c in range(NCORES):
        lo = c * NODES_PC
        owned = np.arange(lo, lo + NODES_PC)
        dc = deg[owned]
        order = np.argsort(-dc, kind="stable")
        perm_owned = owned[order]
        degs_sorted = dc[order]
        tw = np.maximum(degs_sorted[::128][:NW], 1).astype(np.int64)
        cores.append(dict(perm_owned=perm_owned, tw=tw))

    T_w = np.max(np.stack([cc["tw"] for cc in cores]), axis=0)  # [NW]
    TOFF = np.concatenate([[0], np.cumsum(T_w)])                # slot col offsets
    SUMT = int(TOFF[-1])

    chunks = []           # (w, t0, t1, icol0)
    icol = 0
    for w in range(NW):
        t = 0
        while t < T_w[w]:
            t1 = min(t + TC_TILES, int(T_w[w]))
            chunks.append((w, t, t1, icol))
            icol += (t1 - t) * 8
            t += t1 - t
    IDXCOLS = icol

    per_core = []
    for c in range(NCORES):
        cc = cores[c]
        perm_owned = cc["perm_owned"]
        rest = np.setdiff1d(np.arange(N), perm_owned, assume_unique=True)
        perm = np.concatenate([perm_owned, rest])
        perm_pos = np.empty(N, np.int64)
        perm_pos[perm] = np.arange(N)

        emask = (dst >= c * NODES_PC) & (dst < (c + 1) * NODES_PC)
        e_ids = np.nonzero(emask)[0]
        d_loc = perm_pos[dst[e_ids]]                 # 0..6249
        eorder = np.argsort(d_loc, kind="stable")
        e_s = e_ids[eorder]
        ds = d_loc[eorder]
        starts = np.searchsorted(ds, np.arange(NODES_PC))
        t_of = np.arange(len(ds)) - starts[ds]
        w_of = ds // 128
        p_of = ds % 128
        col = TOFF[w_of] + t_of

        src_rel = perm_pos[src[e_s]]
        par = (src_rel & 1).astype(np.float32)

        idx_slot = np.zeros((128, SUMT), np.int16)
        sel = np.zeros((2, 128, SUMT), np.float32)
        ea_slot = np.zeros((128, SUMT, EDGE_F), np.float32)
        idx_slot[p_of, col] = (src_rel >> 1).astype(np.int16)
        sel[0, p_of, col] = 1.0 - par
        sel[1, p_of, col] = par
        ea_slot[p_of, col] = edge_attr[e_s]

        idx16 = np.zeros((128, IDXCOLS), np.int16)
        for (w, t0, t1, ic0) in chunks:
            ncol = (t1 - t0) * 8
            flat = idx_slot[:, TOFF[w] + t0: TOFF[w] + t1].T.reshape(-1)
            wrapped = flat.reshape(-1, 16).T
            idx16[:, ic0: ic0 + ncol] = np.tile(wrapped, (8, 1))

        xT_ext = np.empty((IN_F + 1, N), np.float32)
        xT_ext[:IN_F] = x[perm].T
        xT_ext[IN_F] = 1.0

        per_core.append(dict(
            xT=xT_ext,
            idx16=idx16,
            sel=sel,
            ea=ea_slot.reshape(128, SUMT * EDGE_F),
            perm_owned=perm_owned,
        ))

    wlind = np.concatenate([wlinT.astype(np.float32), dws], axis=1)  # [64, 132]
    common = dict(T_w=T_w, TOFF=TOFF, SUMT=SUMT, chunks=chunks, IDXCOLS=IDXCOLS,
                  wlind=wlind, dext=dext, crep=crep, wrese=wrese)
    return common, per_core


def _build_program(common):
    import concourse.bass as bass
    import concourse.tile as tile
    from concourse import bacc, mybir

    f32 = mybir.dt.float32
    i16 = mybir.dt.int16
    AL = mybir.AluOpType
    AX = mybir.AxisListType
    SUMT, IDXCOLS = common["SUMT"], common["IDXCOLS"]
    T_w, TOFF, chunks = common["T_w"], common["TOFF"], common["chunks"]

    nc = bacc.Bacc("TRN2", target_bir_lowering=False, debug=False,
                   num_devices=NCORES, num_swdge_queues=4)

    xT_d = nc.dram_tensor("xT", [IN_F + 1, N], f32, kind="ExternalInput")
    idx_d = nc.dram_tensor("idx16", [128, IDXCOLS], i16, kind="ExternalInput")
    sel_d = nc.dram_tensor("sel", [2, 128, SUMT], f32, kind="ExternalInput")
    ea_d = nc.dram_tensor("ea", [128, SUMT * EDGE_F], f32, kind="ExternalInput")
    wlin_d = nc.dram_tensor("wlind", [IN_F, 132], f32, kind="ExternalInput")
    dext_d = nc.dram_tensor("dext", [IN_F + 1, HEADS], f32, kind="ExternalInput")
    crep_d = nc.dram_tensor("crep", [128, HEADS * EDGE_F], f32, kind="ExternalInput")
    wrese_d = nc.dram_tensor("wrese", [IN_F + 1, 128], f32, kind="ExternalInput")
    out_d = nc.dram_tensor("out", [WNODES, 128], f32, kind="ExternalOutput")

    with tile.TileContext(nc) as tc, ExitStack() as ctx:
        const = ctx.enter_context(tc.tile_pool(name="const", bufs=1))
        dramp = ctx.enter_context(tc.tile_pool(name="dram", bufs=1, space="DRAM"))
        xp_t = dramp.tile([N // 2, ROWF], f32)

        wlint = const.tile([IN_F, 132], f32)
        nc.sync.dma_start(wlint[:], wlin_d.ap())
        dext_t = const.tile([IN_F + 1, HEADS], f32)
        nc.sync.dma_start(dext_t[:], dext_d.ap())
        crep_t = const.tile([128, HEADS * EDGE_F], f32)
        nc.sync.dma_start(crep_t[:], crep_d.ap())
        wrese_t = const.tile([IN_F + 1, 128], f32)
        nc.sync.dma_start(wrese_t[:], wrese_d.ap())
        xTown = const.tile([IN_F + 1, WNODES], f32)
        nc.sync.dma_start(xTown[:], xT_d.ap()[:, 0:WNODES])
        selL = const.tile([128, SUMT], f32)
        nc.sync.dma_start(selL[:], sel_d.ap()[0])
        selH = const.tile([128, SUMT], f32)
        nc.sync.dma_start(selH[:], sel_d.ap()[1])
        atb = const.tile([128, NW * HEADS], f32)

        # ---- pass-0: gather table ([25000, 320] pair rows) + a_t columns ----
        NBLK = (N + 127) // 128          # 391 node blocks of 128
        GB = 8                           # blocks per batched table write
        SLABW = 12544                    # 98 blocks per slab (slab-aligned groups)
        with tc.tile_pool(name="p0slab", bufs=2) as slabp, \
             tc.tile_pool(name="p0", bufs=3) as p0, \
             tc.tile_pool(name="p0ps", bufs=4, space="PSUM") as p0ps:
            xp_flat = xp_t[:]            # [25000, 320]
            nslab = (N + SLABW - 1) // SLABW
            for sl in range(nslab):
                c0 = sl * SLABW
                cw = min(SLABW, N - c0)
                slab = slabp.tile([IN_F, SLABW], f32, tag="slab")
                nc.sync.dma_start(slab[:, :cw], xT_d.ap()[0:IN_F, c0:c0 + cw])
                b0 = c0 // 128
                bn = (cw + 127) // 128
                for bg in range(b0, b0 + bn, GB):
                    gn = min(GB, b0 + bn - bg)
                    stage = p0.tile([128, GB * 132], f32, tag="stage")
                    for k in range(gn):
                        b = bg + k
                        nb = min(128, N - b * 128)
                        lo = b * 128 - c0
                        if nb < 128:
                            nc.vector.memset(stage[:, k * 132:(k + 1) * 132], 0.0)
                        ps = p0ps.tile([128, 132], f32, tag="ps")
                        nc.tensor.matmul(ps[:nb, :], slab[:, lo:lo + nb],
                                         wlint[:], start=True, stop=True)
                        nc.scalar.copy(stage[:nb, k * 132:(k + 1) * 132], ps[:nb, :])
                    gfull = gn
                    if bg + gn == NBLK and N % 128 != 0:
                        gfull = gn - 1
                    for par in range(2):
                        src = stage[:].rearrange("(r a) c -> a r c", a=2)[par] \
                                      .rearrange("r (k c) -> r k c", c=132)
                        if gfull > 0:
                            dst_xp = xp_flat[64 * bg: 64 * (bg + gfull),
                                             128 * par: 128 * par + 128] \
                                .rearrange("(k r) f -> r k f", k=gfull)
                            nc.sync.dma_start(dst_xp, src[:, :gfull, 0:128])
                            dst_as = xp_flat[64 * bg: 64 * (bg + gfull),
                                             256 + HEADS * par: 256 + HEADS * (par + 1)] \
                                .rearrange("(k r) h -> r k h", k=gfull)
                            nc.sync.dma_start(dst_as, src[:, :gfull, 128:132])
                        if gfull < gn:
                            b = bg + gfull
                            rows = (N - b * 128) // 2     # pair rows in partial block
                            r0 = 64 * b
                            nc.sync.dma_start(
                                xp_flat[r0: r0 + rows, 128 * par: 128 * par + 128],
                                src[:rows, gfull, 0:128])
                            nc.sync.dma_start(
                                xp_flat[r0: r0 + rows,
                                        256 + HEADS * par: 256 + HEADS * (par + 1)],
                                src[:rows, gfull, 128:132])
            for w in range(NW):
                ps2 = p0ps.tile([128, HEADS], f32, tag="ps2")
                nc.tensor.matmul(ps2[:], xTown[:, w * 128:(w + 1) * 128], dext_t[:],
                                 start=True, stop=True)
                nc.scalar.copy(atb[:, w * HEADS:(w + 1) * HEADS], ps2[:])

        # ---- main loop ----
        with tc.tile_pool(name="xsp", bufs=4) as xsp, \
             tc.tile_pool(name="eap", bufs=4) as eap, \
             tc.tile_pool(name="idxp", bufs=4) as idxp, \
             tc.tile_pool(name="scr", bufs=2) as scr, \
             tc.tile_pool(name="sml", bufs=3) as sml, \
             tc.tile_pool(name="rhsp", bufs=3) as rhsp, \
             tc.tile_pool(name="nap", bufs=2) as nap, \
             tc.tile_pool(name="outp", bufs=3) as outp, \
             tc.tile_pool(name="mps", bufs=2, space="PSUM") as mps:

            qrr = 0
            wchunks = {}
            for ch in chunks:
                wchunks.setdefault(ch[0], []).append(ch)

            for w in range(NW):
                res_ps = mps.tile([128, 128], f32, tag="res")
                nc.tensor.matmul(res_ps[:], xTown[:, w * 128:(w + 1) * 128],
                                 wrese_t[:], start=True, stop=True)
                num_acc = nap.tile([128, 132], f32, tag="num")
                first = True
                for (_, t0, t1, ic0) in wchunks[w]:
                    tcn = t1 - t0
                    nidx = tcn * 128
                    scol = int(TOFF[w]) + t0

                    idxc = idxp.tile([128, TC_TILES * 8], i16, tag="idxc")
                    nc.sync.dma_start(idxc[:, :tcn * 8], idx_d.ap()[:, ic0: ic0 + tcn * 8])
                    xs = xsp.tile([128, TC_TILES, ROWF], f32, tag="xs")
                    nc.gpsimd.dma_gather(
                        xs[:, :tcn, :], xp_t[:], idxc[:, :tcn * 8],
                        nidx, nidx, ROWF, single_packet=False, queue_num=qrr % 4)
                    qrr += 1

                    eat = eap.tile([128, TC_TILES * EDGE_F], f32, tag="eat")
                    nc.sync.dma_start(eat[:, :tcn * EDGE_F],
                                      ea_d.ap()[:, scol * EDGE_F: (scol + tcn) * EDGE_F])

                    # a_e: grouped reduce of ea * C
                    prode = scr.tile([128, TC_TILES * HEADS * EDGE_F], f32, tag="prode")
                    ea_bc = eat[:, :tcn * EDGE_F] \
                        .rearrange("p (t k) -> p t k", t=tcn) \
                        .rearrange("p t (a k) -> p t a k", a=1) \
                        .broadcast_to([128, tcn, HEADS, EDGE_F])
                    crep_bc = crep_t[:].rearrange("p (a f) -> p a f", a=1) \
                        .broadcast_to([128, tcn, HEADS * EDGE_F]) \
                        .rearrange("p t (h k) -> p t h k", h=HEADS)
                    prode_v = prode[:, :tcn * HEADS * EDGE_F] \
                        .rearrange("p (t h k) -> p t h k", t=tcn, h=HEADS)
                    prode_g = prode[:, :tcn * HEADS * EDGE_F] \
                        .rearrange("p (g x) -> p g x", x=EDGE_F)
                    ze = sml.tile([128, TC_TILES * HEADS], f32, tag="ze")
                    nc.vector.tensor_tensor(prode_v, ea_bc, crep_bc, op=AL.mult)
                    nc.vector.tensor_reduce(ze[:, :tcn * HEADS], prode_g,
                                            axis=AX.X, op=AL.add)

                    selLb = selL[:, scol: scol + tcn] \
                        .rearrange("p (t a) -> p t a", a=1).broadcast_to([128, tcn, HEADS])
                    selHb = selH[:, scol: scol + tcn] \
                        .rearrange("p (t a) -> p t a", a=1).broadcast_to([128, tcn, HEADS])
                    atbb = atb[:, w * HEADS:(w + 1) * HEADS] \
                        .rearrange("p (a h) -> p a h", a=1).broadcast_to([128, tcn, HEADS])

                    nh = tcn * HEADS
                    # u = as_lo*selL + as_hi*selH + ze + atb  (a_s slices ride the rows)
                    as_lo = xs[:, :tcn, 256:256 + HEADS]
                    as_hi = xs[:, :tcn, 256 + HEADS:256 + 2 * HEADS]
                    t1t = sml.tile([128, TC_TILES * HEADS], f32, tag="t1")
                    t1v = t1t[:, :nh].rearrange("p (t h) -> p t h", t=tcn)
                    nc.vector.tensor_tensor(t1v, as_lo, selLb, op=AL.mult)
                    t2t = sml.tile([128, TC_TILES * HEADS], f32, tag="t2")
                    t2v = t2t[:, :nh].rearrange("p (t h) -> p t h", t=tcn)
                    nc.vector.tensor_tensor(t2v, as_hi, selHb, op=AL.mult)
                    u = sml.tile([128, TC_TILES * HEADS], f32, tag="u")
                    u_v = u[:, :nh].rearrange("p (t h) -> p t h", t=tcn)
                    ze_v = ze[:, :nh].rearrange("p (t h) -> p t h", t=tcn)
                    nc.vector.tensor_tensor(u_v, t1v, t2v, op=AL.add)
                    nc.vector.tensor_tensor(u_v, u_v, ze_v, op=AL.add)
                    nc.vector.tensor_tensor(u_v, u_v, atbb, op=AL.add)
                    lr = sml.tile([128, TC_TILES * HEADS], f32, tag="lr")
                    nc.vector.scalar_tensor_tensor(lr[:, :nh], u[:, :nh], NEG_SLOPE,
                                                   u[:, :nh], op0=AL.mult, op1=AL.max)
                    ev = sml.tile([128, TC_TILES * HEADS], f32, tag="ev")
                    nc.scalar.activation(ev[:, :nh], lr[:, :nh],
                                         mybir.ActivationFunctionType.Exp)
                    ev_v = ev[:, :nh].rearrange("p (t h) -> p t h", t=tcn)

                    rhs = rhsp.tile([128, TC_TILES, 2, 132], f32, tag="rhs")
                    evlo = rhs[:, :tcn, 0, 128:132]
                    evhi = rhs[:, :tcn, 1, 128:132]
                    nc.vector.tensor_tensor(evlo, ev_v, selLb, op=AL.mult)
                    nc.vector.tensor_tensor(evhi, ev_v, selHb, op=AL.mult)
                    evlo_bc = evlo.rearrange("p t (h a) -> p t h a", a=1) \
                                  .broadcast_to([128, tcn, HEADS, OUT_F])
                    evhi_bc = evhi.rearrange("p t (h a) -> p t h a", a=1) \
                                  .broadcast_to([128, tcn, HEADS, OUT_F])
                    msg_lo = rhs[:, :tcn, 0, 0:128].rearrange("p t (h f) -> p t h f", h=HEADS)
                    msg_hi = rhs[:, :tcn, 1, 0:128].rearrange("p t (h f) -> p t h f", h=HEADS)
                    xs_lo4 = xs[:, :tcn, 0:128].rearrange("p t (h f) -> p t h f", h=HEADS)
                    xs_hi4 = xs[:, :tcn, 128:256].rearrange("p t (h f) -> p t h f", h=HEADS)
                    nc.vector.tensor_tensor(msg_lo, xs_lo4, evlo_bc, op=AL.mult)
                    nc.vector.tensor_tensor(msg_hi, xs_hi4, evhi_bc, op=AL.mult)

                    # num += sum over (t, half): contiguous pairwise fold
                    flat = rhs[:].rearrange("p t h f -> p (t h) f")   # [128, 2*TC, 132]
                    n = 2 * tcn
                    while n > 1:
                        k = n // 2
                        nc.vector.tensor_tensor(flat[:, 0:k, :], flat[:, 0:k, :],
                                                flat[:, n - k:n, :], op=AL.add)
                        n -= k
                    if first:
                        nc.vector.tensor_copy(num_acc[:], flat[:, 0, :])
                        first = False
                    else:
                        nc.vector.tensor_tensor(num_acc[:], num_acc[:], flat[:, 0, :],
                                                op=AL.add)

                # ---- window close ----
                dn = outp.tile([128, HEADS], f32, tag="dn")
                nc.vector.tensor_scalar_max(dn[:], num_acc[:, 128:132], 1e-30)
                rec = outp.tile([128, HEADS], f32, tag="rec")
                nc.vector.reciprocal(rec[:], dn[:])
                outw = outp.tile([128, 128], f32, tag="outw")
                outw_v = outw[:].rearrange("p (h f) -> p h f", h=HEADS)
                num_v = num_acc[:, 0:128].rearrange("p (h f) -> p h f", h=HEADS)
                rec_bc = rec[:].rearrange("p (h a) -> p h a", a=1) \
                               .broadcast_to([128, HEADS, OUT_F])
                nc.vector.tensor_tensor(outw_v, num_v, rec_bc, op=AL.mult)
                out2 = outp.tile([128, 128], f32, tag="out2")
                nc.vector.tensor_tensor(out2[:], outw[:], res_ps[:], op=AL.add)
                nc.sync.dma_start(out_d.ap()[w * 128:(w + 1) * 128, :], out2[:])

    nc.compile()
    return nc


def kernel(**inputs):
    from concourse.bass_utils import run_bass_kernel_spmd

    args = {k: np.asarray(v) for k, v in inputs.items()}
    common, per_core = _host_preprocess(
        args["x"], args["edge_index"], args["edge_attr"], args["W_lin"],
        args["w_s"], args["b_s"], args["w_t"], args["b_t"], args["W_edge"],
        args["w_e"], args["b_e"], args["W_res"], args["bias"])

    nc = _build_program(common)

    in_maps = []
    for c in range(NCORES):
        pc = per_core[c]
        in_maps.append({
            "xT": pc["xT"], "idx16": pc["idx16"], "sel": pc["sel"], "ea": pc["ea"],
            "wlind": common["wlind"], "dext": common["dext"],
            "crep": common["crep"], "wrese": common["wrese"],
        })

    res = run_bass_kernel_spmd(nc, in_maps, list(range(NCORES)),
                               trace=bool(os.environ.get("GAT_TRACE")),
                               tmpdir=os.environ.get("GAT_TMPDIR"))
    if os.environ.get("GAT_TRACE"):
        print(f"HW exec time: {res.exec_time_ns} ns")

    out = np.empty((N, HEADS * OUT_F), np.float32)
    for c in range(NCORES):
        out[per_core[c]["perm_owned"]] = res.results[c]["out"][:NODES_PC]
    return out
